# revision 40
# baseline (speedup 1.0000x reference)
"""Trainium2 Bass kernel for 2-layer GAT (nn_GAT_72619307041134).

Strategy (dst-sharded edge parallelism, 8 cores SPMD):
- Nodes sharded into 8 contiguous ranges of 6250; edges sorted by dst and
  sharded by dst range, so each core owns ALL edges of its dst nodes and the
  segment softmax + aggregation need no cross-core reduction.
- Per layer a per-node gather table lives in DRAM with 256 B rows (the
  dma_gather minimum): layer1 rows hold h1 (128 bf16), layer2 rows hold
  h2 (64 bf16 + zero pad).  The per-edge src logits al_src are NOT stored:
  they are recomputed on-chip from the gathered rows (DVE multiply by a
  replicated a_src row + grouped reduce), and the per-edge dst logits al_dst
  are expanded on-chip from a per-window SBUF table via a transposed-indicator
  PE matmul (no per-edge dst gather at all).  This halves layer-1 gather
  bytes and removes the 256 B/edge ald gathers entirely - the SWDGE queues
  (~15 GB/s each, 4 max) are the bottleneck resource.
- Edges are processed in 128-edge chunks (16 chunks = 1 super-chunk):
  dma_gather fetches the src rows (int16 indices; srcs >= SPLIT gather from a
  shifted view of the table; chunks are A/B-pure and A-first within each
  super-chunk); the two gathers of each super-chunk rotate over all 4 SWDGE
  queues.  DVE computes ee = exp(leaky_relu(al_s+al_d)) and
  msg = [ee*h | ee] in bf16; an indicator S0[p,j] = (dst_rel[p]==j) turns the
  per-128-node-window segment-sum into PE matmuls accumulating in PSUM.  The
  transposed indicator S0T (for the al_dst expansion) is built by a
  block-swizzled is_equal followed by a single DVE 32x32 stream-transpose.
  Window epilogue divides by the softmax denominator (alpha = ee/(sum+1e-16);
  the segment-max shift is skipped because logits are bounded by ~4).
- Pad edges have dst_rel=-1 (S0/S0T columns all zero) and gather row 0.
- Between layers one AllGather shares each core's table shard.
- Host preprocessing only sorts/pads/permutes integer indices.
"""

from contextlib import ExitStack

import numpy as np

try:
    import ml_dtypes

    BF16 = ml_dtypes.bfloat16
except Exception:  # pragma: no cover
    BF16 = np.float32

# ---------------------------------------------------------------------------
# config
# ---------------------------------------------------------------------------


class Cfg:
    def __init__(self, N=50000, E=800000, NCORES=8, WIN=128, CHUNK=128, SC=16,
                 SPLIT=32768):
        self.N = N
        self.E = E
        self.NCORES = NCORES
        self.NSH = N // NCORES
        self.WIN = WIN
        self.NWIN = (self.NSH + WIN - 1) // WIN
        self.CHUNK = CHUNK
        self.SC = SC
        self.SPLIT = SPLIT        # src < SPLIT -> table A view, else B view
        self.HEADS = 8
        self.HID = 16
        self.OUT_C = 64
        self.R = 128              # bf16 elems per table row (256 B)
        self.MSG1 = 128 + 8
        self.MSG2 = 64 + 1
        self.NQUEUES = 4


FULL = Cfg()

# ---------------------------------------------------------------------------
# host-side edge preprocessing (indices only)
# ---------------------------------------------------------------------------


def prep_edges(edge_index: np.ndarray, cfg: Cfg):
    """Builds the SPMD-common chunk schedule and per-core index arrays.

    meta: cmap (slot -> (window, first, last)), nA (A-chunks per super-chunk,
    A-first slot order), TC, NG.
    per_core[c]: int16 'src_idx' [128, TC*CHUNK/16] (wrapped: edge i of the
    slot-ordered stream at [i%16, i//16], replicated to all 8 Q7 core pairs),
    bf16 'dst_rel' [128, TC] (rel[p, slot], -1 for pads) and bf16 'rel4'
    [128, TC*4] (quadrant-replicated rel for the transposed-indicator build:
    rel4[32q+r, slot*4+b] = rel[32b+r, slot]).
    """
    N, NSH, WIN, CHUNK, SC = cfg.N, cfg.NSH, cfg.WIN, cfg.CHUNK, cfg.SC
    loops = np.arange(N, dtype=np.int64)
    src = np.concatenate([edge_index[0].astype(np.int64), loops])
    dst = np.concatenate([edge_index[1].astype(np.int64), loops])
    order = np.argsort(dst, kind="stable")
    src = src[order]
    dst = dst[order]
    isA = src < cfg.SPLIT
    core_of = dst // NSH
    w_of = (dst % NSH) // WIN
    cntA = np.zeros((cfg.NCORES, cfg.NWIN), np.int64)
    cntB = np.zeros((cfg.NCORES, cfg.NWIN), np.int64)
    np.add.at(cntA, (core_of[isA], w_of[isA]), 1)
    np.add.at(cntB, (core_of[~isA], w_of[~isA]), 1)
    cpwA = np.ceil(cntA.max(0) / CHUNK).astype(np.int64)
    cpwB = np.ceil(cntB.max(0) / CHUNK).astype(np.int64)
    tc = int((cpwA + cpwB).sum())
    cpwA[-1] += (-tc) % SC
    TC = int((cpwA + cpwB).sum())
    NG = TC // SC

    # global chunk list: window-major, A then B inside a window
    glist = []  # (window, is_A)
    for w in range(cfg.NWIN):
        glist += [(w, True)] * int(cpwA[w]) + [(w, False)] * int(cpwB[w])
    # per-super-chunk stable reorder: A-chunks first
    slot_of = []
    nA = []
    for g in range(NG):
        blk = list(range(g * SC, (g + 1) * SC))
        a = [i for i in blk if glist[i][1]]
        b = [i for i in blk if not glist[i][1]]
        nA.append(len(a))
        slot_of += a + b
    lastslot = {}
    for s, gi in enumerate(slot_of):
        lastslot[glist[gi][0]] = s
    cmap = []
    seen = set()
    inflight = 0
    max_inflight = 0
    for s, gi in enumerate(slot_of):
        w = glist[gi][0]
        first = w not in seen
        seen.add(w)
        last = lastslot[w] == s
        if first:
            inflight += 1
            max_inflight = max(max_inflight, inflight)
        cmap.append((w, first, last))
        if last:
            inflight -= 1

    starts = np.searchsorted(
        dst, (np.arange(0, N, NSH)[:, None] + np.arange(0, NSH, WIN)[None, :]))
    flat_starts = list(starts.ravel()) + [len(dst)]
    per_core = []
    for c in range(cfg.NCORES):
        s_by_g = np.zeros((len(glist), CHUNK), np.int64)
        r_by_g = np.full((len(glist), CHUNK), -1, np.int64)
        gi = 0
        for w in range(cfg.NWIN):
            i = c * cfg.NWIN + w
            s0, s1 = flat_starts[i], flat_starts[i + 1]
            sl = src[s0:s1]
            dl = dst[s0:s1]
            il = isA[s0:s1]
            for grp in (True, False):
                m = il == grp
                se = sl[m]
                de = dl[m]
                nch = int(cpwA[w]) if grp else int(cpwB[w])
                buf_s = np.zeros(nch * CHUNK, np.int64)
                buf_r = np.full(nch * CHUNK, -1, np.int64)
                buf_s[:len(se)] = se - (0 if grp else cfg.SPLIT)
                buf_r[:len(se)] = de - (c * NSH + w * WIN)
                s_by_g[gi:gi + nch] = buf_s.reshape(nch, CHUNK)
                r_by_g[gi:gi + nch] = buf_r.reshape(nch, CHUNK)
                gi += nch
        svals = s_by_g[slot_of]   # [TC, 128] slot-ordered
        rvals = r_by_g[slot_of]

        def wrap(vals):
            st = vals.reshape(-1)   # stream i = slot*128 + p
            n = st.shape[0]
            out = np.zeros((16, n // 16), np.int16)
            idx = np.arange(n)
            out[idx % 16, idx // 16] = st
            return np.tile(out, (8, 1))

        relT = np.ascontiguousarray(rvals.T).astype(np.float32)  # [128, TC]
        # rel4[32q+r, t*4+b] = relT[32b+r, t]  (pre-swizzled rel for the
        # block-transposed indicator build)
        rel4q = relT.reshape(4, 32, TC).transpose(1, 2, 0).reshape(32, TC * 4)
        rel4 = np.tile(rel4q, (4, 1))
        per_core.append(dict(
            src_idx=wrap(svals),
            dst_rel=relT.astype(BF16),
            rel4=np.ascontiguousarray(rel4).astype(BF16),
        ))
    meta = dict(cmap=cmap, nA=nA, TC=TC, NG=NG, max_inflight=max_inflight)
    return meta, per_core


def host_tensors(inputs, cfg: Cfg):
    x = np.ascontiguousarray(inputs["x"], np.float32)
    W1 = np.ascontiguousarray(inputs["W1"], np.float32)
    a1s = np.asarray(inputs["a1_src"], np.float32)
    a1d = np.asarray(inputs["a1_dst"], np.float32)
    W2 = np.ascontiguousarray(inputs["W2"], np.float32)
    a2s = np.asarray(inputs["a2_src"], np.float32).reshape(1, -1)
    a2d = np.asarray(inputs["a2_dst"], np.float32).reshape(1, -1)
    b1 = np.asarray(inputs["b1"], np.float32)
    b2 = np.asarray(inputs["b2"], np.float32)
    H, HID = cfg.HEADS, cfg.HID
    A1d = np.zeros((H * HID, H), np.float32)
    for h in range(H):
        A1d[h * HID:(h + 1) * HID, h] = a1d[h]
    RHS2 = np.concatenate([W2 @ a2d.T, W2], 1).astype(np.float32)  # [128, 65]
    a1s_rep = np.tile(a1s.reshape(1, -1), (128, 1)).astype(BF16)   # [128,128]
    a2s_flat = np.zeros((1, 128), np.float32)
    a2s_flat[0, :cfg.OUT_C] = a2s[0]
    a2s_rep = np.tile(a2s_flat, (128, 1)).astype(BF16)             # [128,128]
    ident = np.eye(128, dtype=np.float32)
    iota_row = np.tile(np.arange(128, dtype=np.float32).reshape(1, -1),
                       (128, 1)).astype(BF16)                  # [p,j]=j
    p = np.arange(128)
    Qt = ((p // 32 * 32)[:, None] + np.arange(32)[None, :]).astype(
        np.float32).astype(BF16)                               # [128,32]
    shared = dict(W1=W1.astype(BF16), A1d=A1d.astype(BF16), RHS2=RHS2,
                  a1s_rep=a1s_rep, a2s_rep=a2s_rep, ident=ident,
                  iota_row=iota_row, Qt=Qt,
                  b1rep=np.tile(b1.reshape(1, -1), (128, 1)),
                  b2rep=np.tile(b2.reshape(1, -1), (128, 1)))
    xT = [np.ascontiguousarray(x[c * cfg.NSH:(c + 1) * cfg.NSH].T).astype(BF16)
          for c in range(cfg.NCORES)]
    has_bias = bool(np.any(b1) or np.any(b2))
    return shared, xT, has_bias


# ---------------------------------------------------------------------------
# device kernel emission
# ---------------------------------------------------------------------------


def _ap(base, free_dims, extra_off=0):
    """Replace the free dims of a [P, ...] AP (keep partition dim)."""
    import concourse.bass as bass

    return bass.AP(base.tensor, base.offset + extra_off,
                   [list(base.ap[0])] + [list(d) for d in free_dims])


def emit_gat(tc, out_ap, ins, meta, cfg: Cfg, has_bias=False):
    import concourse.bass as bass  # noqa: F401
    from concourse import mybir

    nc = tc.nc
    f32 = mybir.dt.float32
    bf16 = mybir.dt.bfloat16
    i16 = mybir.dt.int16
    AF = mybir.ActivationFunctionType
    OP = mybir.AluOpType
    AX = mybir.AxisListType
    N, NSH, WIN, NWIN, SC = cfg.N, cfg.NSH, cfg.WIN, cfg.NWIN, cfg.SC
    TC, NG = meta["TC"], meta["NG"]
    cmap, nA = meta["cmap"], meta["nA"]
    NQ = cfg.NQUEUES
    R = cfg.R
    M1, M2 = cfg.MSG1, cfg.MSG2
    NIW = TC * cfg.CHUNK // 16

    ctx = ExitStack()
    with ctx:
        dram = ctx.enter_context(tc.tile_pool(name="dram", bufs=1, space="DRAM"))
        consts = ctx.enter_context(tc.tile_pool(name="consts", bufs=1))

        t1shard = dram.tile([NSH, R], bf16)
        t1full = dram.tile([N, R], bf16, addr_space="Shared")
        t2shard = dram.tile([NSH, R], bf16)
        t2full = dram.tile([N, R], bf16, addr_space="Shared")

        # ------- constants into SBUF -------
        W1_sb = consts.tile([128, 128], bf16)
        A1d_sb = consts.tile([128, 8], bf16)
        RHS2_sb = consts.tile([128, 65], f32)
        a1s_sb = consts.tile([128, 128], bf16)
        a2s_sb = consts.tile([128, 128], bf16)
        id_sb = consts.tile([128, 128], f32)
        iota_sb = consts.tile([128, 128], bf16)
        Qt_sb = consts.tile([128, 32], bf16)
        nc.sync.dma_start(W1_sb[:], ins["W1"][:])
        nc.sync.dma_start(A1d_sb[:], ins["A1d"][:])
        nc.sync.dma_start(RHS2_sb[:], ins["RHS2"][:])
        nc.sync.dma_start(a1s_sb[:], ins["a1s_rep"][:])
        nc.sync.dma_start(a2s_sb[:], ins["a2s_rep"][:])
        nc.sync.dma_start(id_sb[:], ins["ident"][:])
        nc.sync.dma_start(iota_sb[:], ins["iota_row"][:])
        nc.sync.dma_start(Qt_sb[:], ins["Qt"][:])
        src_sb = consts.tile([128, NIW], i16)
        rel_sb = consts.tile([128, TC], bf16)
        rel4_sb = consts.tile([128, TC * 4], bf16)
        nc.sync.dma_start(src_sb[:], ins["src_idx"][:])
        nc.sync.dma_start(rel_sb[:], ins["dst_rel"][:])
        nc.sync.dma_start(rel4_sb[:], ins["rel4"][:])
        if has_bias:
            b1rep_sb = consts.tile([128, 128], f32)
            b2rep_sb = consts.tile([128, 64], f32)
            nc.sync.dma_start(b1rep_sb[:], ins["b1rep"][:])
            nc.sync.dma_start(b2rep_sb[:], ins["b2rep"][:])

        # per-window dst logits, resident in SBUF for the whole kernel
        # (memset: the last window writes only 106 partitions; the matmul
        # contracts all 128, and 0 * garbage-Inf would poison the PSUM)
        aldW_sb = consts.tile([128, NWIN * 8], bf16)
        ald2_sb = consts.tile([128, NWIN], bf16)
        nc.vector.memset(aldW_sb[:], 0.0)
        nc.vector.memset(ald2_sb[:], 0.0)

        # ------- stage A: h1 rows -> table1, al_dst -> aldW_sb -------
        with tc.tile_pool(name="stageA", bufs=1) as sa, \
             tc.tile_pool(name="stageApsum", bufs=2, space="PSUM") as sap, \
             tc.tile_pool(name="rows", bufs=3) as rows:
            xT_sb = sa.tile([128, NSH], bf16)
            nc.sync.dma_start(xT_sb[:], ins["xT"][:])
            h1T_sb = sa.tile([128, NSH], bf16)
            al_sb = sa.tile([8, NSH], f32)
            nt = (NSH + 511) // 512
            for j in range(nt):
                w0 = j * 512
                w1 = min(NSH, w0 + 512)
                ph = sap.tile([128, 512], f32, tag="ph")
                nc.tensor.matmul(ph[:, : w1 - w0], W1_sb[:], xT_sb[:, w0:w1],
                                 start=True, stop=True)
                nc.vector.tensor_copy(h1T_sb[:, w0:w1], ph[:, : w1 - w0])
            for j in range(nt):
                w0 = j * 512
                w1 = min(NSH, w0 + 512)
                pa = sap.tile([8, 512], f32, tag="pa")
                nc.tensor.matmul(pa[:, : w1 - w0], A1d_sb[:], h1T_sb[:, w0:w1],
                                 start=True, stop=True)
                nc.vector.tensor_copy(al_sb[:, w0:w1], pa[:, : w1 - w0])

            for w in range(NWIN):
                w0 = w * WIN
                wn = min(WIN, NSH - w0)
                hp = sap.tile([128, 128], f32, tag="hp")
                nc.tensor.matmul(hp[:wn, :], xT_sb[:, w0:w0 + wn], W1_sb[:],
                                 start=True, stop=True)
                at = sap.tile([128, 8], f32, tag="at")
                nc.tensor.transpose(at[:wn, :], al_sb[:, w0:w0 + wn], id_sb[:8, :8])
                rowt = rows.tile([128, R], bf16, tag="rowt")
                nc.scalar.copy(rowt[:wn, :], hp[:wn, :])
                nc.sync.dma_start(t1shard.opt()[w0:w0 + wn, :], rowt[:wn, :])
                nc.vector.tensor_copy(aldW_sb[:wn, w * 8:(w + 1) * 8], at[:wn, :])

        from concourse import library_config

        nc.gpsimd.load_library(library_config.mlp)

        nc.gpsimd.collective_compute(
            "AllGather", mybir.AluOpType.bypass,
            replica_groups=[list(range(cfg.NCORES))],
            ins=[t1shard.opt()], outs=[t1full.opt()],
        )

        # ------- edge layers -------
        def edge_layer(table_full, nh, chper, msgc, a_rep_sb, ald_ap_of_w,
                       epilogue, qoff):
            with tc.tile_pool(name="gbuf", bufs=4) as gpool, \
                 tc.tile_pool(name="ebig", bufs=2) as epool, \
                 tc.tile_pool(name="emsg", bufs=3) as mp, \
                 tc.tile_pool(name="epsum", bufs=meta["max_inflight"] + 1,
                              space="PSUM") as pp, \
                 tc.tile_pool(name="aldpsum", bufs=2, space="PSUM") as app, \
                 tc.tile_pool(name="esmall", bufs=3) as spool:
                pw_by_w = {}
                for g in range(NG):
                    na = nA[g]
                    gb = gpool.tile([128, SC * R], bf16, tag="gb")
                    gb3 = gb[:].rearrange("p (k e) -> p k e", k=SC)
                    c0 = g * SC * 8
                    for grp in range(2):
                        nch = na if grp == 0 else SC - na
                        if nch == 0:
                            continue
                        ksl = slice(0, na) if grp == 0 else slice(na, SC)
                        csl = (slice(c0, c0 + na * 8) if grp == 0
                               else slice(c0 + na * 8, c0 + SC * 8))
                        tbl = (table_full.opt() if grp == 0
                               else table_full.opt()[cfg.SPLIT:N, :])
                        nc.gpsimd.dma_gather(
                            gb3[:, ksl, :], tbl, src_sb[:, csl],
                            num_idxs=nch * 128, num_idxs_reg=nch * 128,
                            elem_size=R, single_packet=False,
                            queue_num=(g + qoff + 2 * grp) % NQ,
                        )
                    # src logits recomputed from the gathered rows
                    prod = epool.tile([128, SC * 128], bf16, tag="prod")
                    nc.vector.tensor_tensor(
                        _ap(prod[:], [[128, SC], [1, 128]]),
                        _ap(gb[:], [[128, SC], [1, 128]]),
                        _ap(a_rep_sb[:], [[0, SC], [1, 128]]),
                        OP.mult,
                    )
                    als = spool.tile([128, SC * nh], f32, tag="als")
                    nc.vector.tensor_reduce(
                        _ap(als[:], [[nh, SC], [1, nh]]),
                        _ap(prod[:], [[128, SC], [chper, nh], [1, chper]]),
                        AX.X, OP.add,
                    )
                    # transposed indicator S0T via block-swizzled is_equal +
                    # 32x32 stream transpose
                    xsw = epool.tile([128, SC * 128], bf16, tag="xsw")
                    nc.vector.tensor_tensor(
                        _ap(xsw[:], [[128, SC], [32, 4], [1, 32]]),
                        _ap(Qt_sb[:], [[0, SC], [0, 4], [1, 32]]),
                        _ap(rel4_sb[:, g * SC * 4:(g + 1) * SC * 4],
                            [[4, SC], [1, 4], [0, 32]]),
                        OP.is_equal,
                    )
                    s0T = epool.tile([128, SC * 128], bf16, tag="s0T")
                    nc.vector.transpose(s0T[:], xsw[:])
                    # per-edge dst logits: aldps[p, h] = sum_j S0T[j,p]*aldW[j,h]
                    aldps = app.tile([128, SC * nh], f32, tag="aldps")
                    for k in range(SC):
                        w = cmap[g * SC + k][0]
                        nc.tensor.matmul(
                            aldps[:, k * nh:(k + 1) * nh],
                            s0T[:, k * 128:(k + 1) * 128],
                            ald_ap_of_w(w),
                            start=True, stop=True,
                        )
                    lg = spool.tile([128, SC * nh], f32, tag="lg")
                    nc.vector.tensor_tensor(lg[:], als[:], aldps[:], OP.add)
                    lgm = spool.tile([128, SC * nh], f32, tag="lgm")
                    nc.vector.scalar_tensor_tensor(
                        lgm[:], lg[:], 0.2, lg[:], OP.mult, OP.max)
                    # exp with built-in replication across the channel block so
                    # the msg multiply runs all-step-1 (DVE 2x mode)
                    B = chper * nh
                    eew = epool.tile([128, SC * B], bf16, tag="eew")
                    nc.scalar.activation(
                        _ap(eew[:], [[B, SC], [1, B]]),
                        _ap(lgm[:], [[nh, SC], [1, nh], [0, chper]]),
                        AF.Exp)
                    msg = mp.tile([128, SC * msgc], bf16, tag="msg")
                    nc.vector.tensor_tensor(
                        _ap(msg[:], [[msgc, SC], [1, B]]),
                        _ap(gb[:], [[R, SC], [1, B]]),
                        _ap(eew[:], [[B, SC], [1, B]]),
                        OP.mult,
                    )
                    nc.scalar.copy(
                        _ap(msg[:], [[msgc, SC], [1, nh]], msgc - nh),
                        _ap(eew[:], [[B, SC], [chper, nh]]),
                    )
                    s0 = epool.tile([128, SC * 128], bf16, tag="s0")
                    nc.vector.tensor_tensor(
                        _ap(s0[:], [[128, SC], [1, 128]]),
                        _ap(iota_sb[:], [[0, SC], [1, 128]]),
                        _ap(rel_sb[:, g * SC:(g + 1) * SC], [[1, SC], [0, 128]]),
                        OP.is_equal,
                    )
                    for k in range(SC):
                        kk = g * SC + k
                        w, first, last = cmap[kk]
                        if first:
                            pw_by_w[w] = pp.tile([128, msgc], f32, tag="pw", name="pw")
                        pw = pw_by_w[w]
                        nc.tensor.matmul(
                            pw[:], s0[:, k * 128:(k + 1) * 128],
                            msg[:, k * msgc:(k + 1) * msgc],
                            start=first, stop=last,
                        )
                        if last:
                            epilogue(w, pw_by_w.pop(w))

        # ---- L1 ----
        with tc.tile_pool(name="epi1", bufs=2) as hq, \
             tc.tile_pool(name="epi1p", bufs=1, space="PSUM") as hpp:
            def epi1(w, pw):
                w0 = w * WIN
                wn = min(WIN, NSH - w0)
                dn = hq.tile([128, 8], f32, tag="dn")
                nc.vector.tensor_scalar(dn[:], pw[:, 128:136], 1e-16, None, OP.add)
                rcp = hq.tile([128, 8], f32, tag="rcp")
                nc.vector.reciprocal(rcp[:], dn[:])
                hb = hq.tile([128, 128], f32, tag="hb")
                nc.vector.tensor_tensor(
                    _ap(hb[:], [[16, 8], [1, 16]]),
                    _ap(pw[:], [[16, 8], [1, 16]]),
                    _ap(rcp[:], [[1, 8], [0, 16]]),
                    OP.mult,
                )
                if has_bias:
                    nc.vector.tensor_tensor(hb[:], hb[:], b1rep_sb[:], OP.add)
                nc.scalar.activation(hb[:], hb[:], AF.Relu)
                tp = hpp.tile([128, 128], f32, tag="tp")
                nc.tensor.transpose(tp[:], hb[:], id_sb[:])
                tH = hq.tile([128, 128], f32, tag="tH")
                nc.vector.tensor_copy(tH[:], tp[:])
                p2 = hpp.tile([128, 65], f32, tag="p2")
                nc.tensor.matmul(p2[:], tH[:], RHS2_sb[:], start=True, stop=True)
                t2b = hq.tile([128, R], bf16, tag="t2b")
                nc.scalar.copy(t2b[:wn, 0:64], p2[:wn, 1:65])
                nc.vector.memset(t2b[:wn, 64:R], 0.0)
                nc.sync.dma_start(t2shard.opt()[w0:w0 + wn, :], t2b[:wn, :])
                nc.vector.tensor_copy(ald2_sb[:wn, w:w + 1], p2[:wn, 0:1])

            edge_layer(t1full, 8, 16, M1, a1s_sb,
                       lambda w: aldW_sb[:, w * 8:(w + 1) * 8], epi1, 0)

        nc.gpsimd.collective_compute(
            "AllGather", mybir.AluOpType.bypass,
            replica_groups=[list(range(cfg.NCORES))],
            ins=[t2shard.opt()], outs=[t2full.opt()],
        )

        # ---- L2 ----
        with tc.tile_pool(name="epi2", bufs=2) as oq:
            def epi2(w, pw):
                w0 = w * WIN
                wn = min(WIN, NSH - w0)
                dn2 = oq.tile([128, 1], f32, tag="dn2")
                nc.vector.tensor_scalar(dn2[:], pw[:, 64:65], 1e-16, None, OP.add)
                rcp2 = oq.tile([128, 1], f32, tag="rcp2")
                nc.vector.reciprocal(rcp2[:], dn2[:])
                ob = oq.tile([128, 64], f32, tag="ob")
                nc.vector.tensor_scalar(ob[:], pw[:, 0:64], rcp2[:], None, OP.mult)
                if has_bias:
                    nc.vector.tensor_tensor(ob[:], ob[:], b2rep_sb[:], OP.add)
                nc.sync.dma_start(out_ap[w0:w0 + wn, :], ob[:wn, :])

            edge_layer(t2full, 1, 64, M2, a2s_sb,
                       lambda w: ald2_sb[:, w:w + 1], epi2, 1)


# ---------------------------------------------------------------------------
# SPMD build + run
# ---------------------------------------------------------------------------

_CACHE = {}


def _build(meta, cfg: Cfg, has_bias: bool):
    key = (tuple(meta["cmap"]), tuple(meta["nA"]), cfg.N, cfg.NCORES, has_bias)
    if key in _CACHE:
        return _CACHE[key]
    import concourse.tile as tile
    from concourse import bacc, mybir

    f32 = mybir.dt.float32
    bf16 = mybir.dt.bfloat16
    i16 = mybir.dt.int16
    TC = meta["TC"]
    NIW = TC * cfg.CHUNK // 16
    nc = bacc.Bacc("TRN2", target_bir_lowering=False, debug=False,
                   num_devices=cfg.NCORES,
                   num_swdge_queues=cfg.NQUEUES)
    ins = {}

    def di(name, shape, dt=f32):
        ins[name] = nc.dram_tensor(name, shape, dt, kind="ExternalInput").ap()

    di("xT", [128, cfg.NSH], bf16)
    di("W1", [128, 128], bf16)
    di("A1d", [128, 8], bf16)
    di("RHS2", [128, 65])
    di("a1s_rep", [128, 128], bf16)
    di("a2s_rep", [128, 128], bf16)
    di("ident", [128, 128])
    di("iota_row", [128, 128], bf16)
    di("Qt", [128, 32], bf16)
    di("src_idx", [128, NIW], i16)
    di("dst_rel", [128, TC], bf16)
    di("rel4", [128, TC * 4], bf16)
    if has_bias:
        di("b1rep", [128, 128])
        di("b2rep", [128, 64])
    out = nc.dram_tensor("out", [cfg.NSH, cfg.OUT_C], f32, kind="ExternalOutput").ap()

    with tile.TileContext(nc) as tc:
        emit_gat(tc, out, ins, meta, cfg, has_bias)
    nc.compile()
    _CACHE[key] = nc
    return nc


def kernel(**inputs) -> np.ndarray:
    out, _ = _run(inputs)
    return out


def _run(inputs, **run_kwargs):
    cfg = FULL
    inputs = {k: np.asarray(v) for k, v in inputs.items()}
    edge_index = inputs["edge_index"].astype(np.int64)
    meta, per_core = prep_edges(edge_index, cfg)
    shared, xT, has_bias = host_tensors(inputs, cfg)
    nc = _build(meta, cfg, has_bias)

    from concourse.bass_utils import run_bass_kernel_spmd

    in_maps = []
    for c in range(cfg.NCORES):
        m = {k: shared[k] for k in ("W1", "A1d", "RHS2", "a1s_rep", "a2s_rep",
                                    "ident", "iota_row", "Qt")}
        if has_bias:
            m["b1rep"] = shared["b1rep"]
            m["b2rep"] = shared["b2rep"]
        m["xT"] = xT[c]
        m.update(per_core[c])
        in_maps.append(m)
    res = run_bass_kernel_spmd(nc, in_maps, core_ids=list(range(cfg.NCORES)),
                               **run_kwargs)
    out = np.concatenate([res.results[c]["out"] for c in range(cfg.NCORES)], 0)
    return out.astype(np.float32), res


# revision 43
# speedup vs baseline: 1.0321x; 1.0321x over previous
"""Trainium2 Bass kernel for 2-layer GAT (nn_GAT_72619307041134).

Strategy (dst-sharded edge parallelism, 8 cores SPMD):
- Nodes sharded into 8 contiguous ranges of 6250; edges sorted by dst and
  sharded by dst range, so each core owns ALL edges of its dst nodes and the
  segment softmax + aggregation need no cross-core reduction.
- Per layer a per-node gather table lives in DRAM with 256 B rows (the
  dma_gather minimum): layer1 rows hold h1 (128 bf16), layer2 rows hold
  h2 (64 bf16 + zero pad).  The per-edge src logits al_src are NOT stored:
  they are recomputed on-chip from the gathered rows (DVE multiply by a
  replicated a_src row + grouped reduce), and the per-edge dst logits al_dst
  are expanded on-chip from a per-window SBUF table via a transposed-indicator
  PE matmul (no per-edge dst gather at all).  This halves layer-1 gather
  bytes and removes the 256 B/edge ald gathers entirely - the SWDGE gather
  queues (~40-60 GB/s aggregate drain, 4 queues max) are the bottleneck
  resource, with DVE a close second; both run ~92% busy in the edge phases.
- Edges are processed in 128-edge chunks (16 chunks = 1 super-chunk):
  dma_gather fetches the src rows (int16 indices; srcs >= SPLIT gather from a
  shifted view of the table; chunks are A/B-pure and A-first within each
  super-chunk); the two gathers of each super-chunk rotate over all 4 SWDGE
  queues.  DVE computes the logits; ACT exp writes a channel-replicated eew
  tile (step-0 input AP, step-1 output) so the big msg = ee*h multiply runs
  as an all-step-1 bf16 tensor_tensor in the DVE 2x mode.  An indicator
  S0[p,j] = (dst_rel[p]==j) turns the per-128-node-window segment-sum into PE
  matmuls accumulating in PSUM (bf16 operands -> FWL weight loads).  The
  transposed indicator S0T (for the al_dst expansion) is built by a
  block-swizzled is_equal followed by a single DVE 32x32 stream-transpose.
  Window epilogue divides by the softmax denominator (alpha = ee/(sum+1e-16);
  the segment-max shift is skipped because logits are bounded by ~4).
- Pad edges have dst_rel=-1 (S0/S0T columns all zero) and gather row 0.
- Between layers one AllGather shares each core's table shard.
- Host preprocessing only sorts/pads/permutes integer indices.
"""

from contextlib import ExitStack

import numpy as np

try:
    import ml_dtypes

    BF16 = ml_dtypes.bfloat16
except Exception:  # pragma: no cover
    BF16 = np.float32

# ---------------------------------------------------------------------------
# config
# ---------------------------------------------------------------------------


class Cfg:
    def __init__(self, N=50000, E=800000, NCORES=8, WIN=128, CHUNK=128, SC=16,
                 SPLIT=32768):
        self.N = N
        self.E = E
        self.NCORES = NCORES
        self.NSH = N // NCORES
        self.WIN = WIN
        self.NWIN = (self.NSH + WIN - 1) // WIN
        self.CHUNK = CHUNK
        self.SC = SC
        self.SPLIT = SPLIT        # src < SPLIT -> table A view, else B view
        self.HEADS = 8
        self.HID = 16
        self.OUT_C = 64
        self.R = 128              # bf16 elems per table row (256 B)
        self.MSG1 = 128 + 8
        self.MSG2 = 64 + 1
        self.NQUEUES = 4


FULL = Cfg()

# ---------------------------------------------------------------------------
# host-side edge preprocessing (indices only)
# ---------------------------------------------------------------------------


def prep_edges(edge_index: np.ndarray, cfg: Cfg):
    """Builds the SPMD-common chunk schedule and per-core index arrays.

    meta: cmap (slot -> (window, first, last)), nA (A-chunks per super-chunk,
    A-first slot order), TC, NG.
    per_core[c]: int16 'src_idx' [128, TC*CHUNK/16] (wrapped: edge i of the
    slot-ordered stream at [i%16, i//16], replicated to all 8 Q7 core pairs),
    bf16 'dst_rel' [128, TC] (rel[p, slot], -1 for pads) and bf16 'rel4'
    [128, TC*4] (quadrant-replicated rel for the transposed-indicator build:
    rel4[32q+r, slot*4+b] = rel[32b+r, slot]).
    """
    N, NSH, WIN, CHUNK, SC = cfg.N, cfg.NSH, cfg.WIN, cfg.CHUNK, cfg.SC
    loops = np.arange(N, dtype=np.int64)
    src = np.concatenate([edge_index[0].astype(np.int64), loops])
    dst = np.concatenate([edge_index[1].astype(np.int64), loops])
    order = np.argsort(dst, kind="stable")
    src = src[order]
    dst = dst[order]
    isA = src < cfg.SPLIT
    core_of = dst // NSH
    w_of = (dst % NSH) // WIN
    cntA = np.zeros((cfg.NCORES, cfg.NWIN), np.int64)
    cntB = np.zeros((cfg.NCORES, cfg.NWIN), np.int64)
    np.add.at(cntA, (core_of[isA], w_of[isA]), 1)
    np.add.at(cntB, (core_of[~isA], w_of[~isA]), 1)
    cpwA = np.ceil(cntA.max(0) / CHUNK).astype(np.int64)
    cpwB = np.ceil(cntB.max(0) / CHUNK).astype(np.int64)
    tc = int((cpwA + cpwB).sum())
    cpwA[-1] += (-tc) % SC
    TC = int((cpwA + cpwB).sum())
    NG = TC // SC

    # global chunk list: window-major, A then B inside a window
    glist = []  # (window, is_A)
    for w in range(cfg.NWIN):
        glist += [(w, True)] * int(cpwA[w]) + [(w, False)] * int(cpwB[w])
    # per-super-chunk stable reorder: A-chunks first
    slot_of = []
    nA = []
    for g in range(NG):
        blk = list(range(g * SC, (g + 1) * SC))
        a = [i for i in blk if glist[i][1]]
        b = [i for i in blk if not glist[i][1]]
        nA.append(len(a))
        slot_of += a + b
    lastslot = {}
    for s, gi in enumerate(slot_of):
        lastslot[glist[gi][0]] = s
    cmap = []
    seen = set()
    inflight = 0
    max_inflight = 0
    for s, gi in enumerate(slot_of):
        w = glist[gi][0]
        first = w not in seen
        seen.add(w)
        last = lastslot[w] == s
        if first:
            inflight += 1
            max_inflight = max(max_inflight, inflight)
        cmap.append((w, first, last))
        if last:
            inflight -= 1

    starts = np.searchsorted(
        dst, (np.arange(0, N, NSH)[:, None] + np.arange(0, NSH, WIN)[None, :]))
    flat_starts = list(starts.ravel()) + [len(dst)]
    per_core = []
    for c in range(cfg.NCORES):
        s_by_g = np.zeros((len(glist), CHUNK), np.int64)
        r_by_g = np.full((len(glist), CHUNK), -1, np.int64)
        gi = 0
        for w in range(cfg.NWIN):
            i = c * cfg.NWIN + w
            s0, s1 = flat_starts[i], flat_starts[i + 1]
            sl = src[s0:s1]
            dl = dst[s0:s1]
            il = isA[s0:s1]
            for grp in (True, False):
                m = il == grp
                se = sl[m]
                de = dl[m]
                nch = int(cpwA[w]) if grp else int(cpwB[w])
                buf_s = np.zeros(nch * CHUNK, np.int64)
                buf_r = np.full(nch * CHUNK, -1, np.int64)
                buf_s[:len(se)] = se - (0 if grp else cfg.SPLIT)
                buf_r[:len(se)] = de - (c * NSH + w * WIN)
                s_by_g[gi:gi + nch] = buf_s.reshape(nch, CHUNK)
                r_by_g[gi:gi + nch] = buf_r.reshape(nch, CHUNK)
                gi += nch
        svals = s_by_g[slot_of]   # [TC, 128] slot-ordered
        rvals = r_by_g[slot_of]

        def wrap(vals):
            st = vals.reshape(-1)   # stream i = slot*128 + p
            n = st.shape[0]
            out = np.zeros((16, n // 16), np.int16)
            idx = np.arange(n)
            out[idx % 16, idx // 16] = st
            return np.tile(out, (8, 1))

        relT = np.ascontiguousarray(rvals.T).astype(np.float32)  # [128, TC]
        # rel4[32q+r, t*4+b] = relT[32b+r, t]  (pre-swizzled rel for the
        # block-transposed indicator build)
        rel4q = relT.reshape(4, 32, TC).transpose(1, 2, 0).reshape(32, TC * 4)
        rel4 = np.tile(rel4q, (4, 1))
        per_core.append(dict(
            src_idx=wrap(svals),
            dst_rel=relT.astype(BF16),
            rel4=np.ascontiguousarray(rel4).astype(BF16),
        ))
    meta = dict(cmap=cmap, nA=nA, TC=TC, NG=NG, max_inflight=max_inflight)
    return meta, per_core


def host_tensors(inputs, cfg: Cfg):
    x = np.ascontiguousarray(inputs["x"], np.float32)
    W1 = np.ascontiguousarray(inputs["W1"], np.float32)
    a1s = np.asarray(inputs["a1_src"], np.float32)
    a1d = np.asarray(inputs["a1_dst"], np.float32)
    W2 = np.ascontiguousarray(inputs["W2"], np.float32)
    a2s = np.asarray(inputs["a2_src"], np.float32).reshape(1, -1)
    a2d = np.asarray(inputs["a2_dst"], np.float32).reshape(1, -1)
    b1 = np.asarray(inputs["b1"], np.float32)
    b2 = np.asarray(inputs["b2"], np.float32)
    H, HID = cfg.HEADS, cfg.HID
    A1d = np.zeros((H * HID, H), np.float32)
    for h in range(H):
        A1d[h * HID:(h + 1) * HID, h] = a1d[h]
    RHS2 = np.concatenate([W2 @ a2d.T, W2], 1).astype(np.float32)  # [128, 65]
    a1s_rep = np.tile(a1s.reshape(1, -1), (128, 1)).astype(BF16)   # [128,128]
    a2s_flat = np.zeros((1, 128), np.float32)
    a2s_flat[0, :cfg.OUT_C] = a2s[0]
    a2s_rep = np.tile(a2s_flat, (128, 1)).astype(BF16)             # [128,128]
    ident = np.eye(128, dtype=np.float32)
    iota_row = np.tile(np.arange(128, dtype=np.float32).reshape(1, -1),
                       (128, 1)).astype(BF16)                  # [p,j]=j
    p = np.arange(128)
    Qt = ((p // 32 * 32)[:, None] + np.arange(32)[None, :]).astype(
        np.float32).astype(BF16)                               # [128,32]
    shared = dict(W1=W1.astype(BF16), A1d=A1d.astype(BF16), RHS2=RHS2,
                  a1s_rep=a1s_rep, a2s_rep=a2s_rep, ident=ident,
                  iota_row=iota_row, Qt=Qt,
                  b1rep=np.tile(b1.reshape(1, -1), (128, 1)),
                  b2rep=np.tile(b2.reshape(1, -1), (128, 1)))
    xT = [np.ascontiguousarray(x[c * cfg.NSH:(c + 1) * cfg.NSH].T).astype(BF16)
          for c in range(cfg.NCORES)]
    has_bias = bool(np.any(b1) or np.any(b2))
    return shared, xT, has_bias


# ---------------------------------------------------------------------------
# device kernel emission
# ---------------------------------------------------------------------------


def _ap(base, free_dims, extra_off=0):
    """Replace the free dims of a [P, ...] AP (keep partition dim)."""
    import concourse.bass as bass

    return bass.AP(base.tensor, base.offset + extra_off,
                   [list(base.ap[0])] + [list(d) for d in free_dims])


def emit_gat(tc, out_ap, ins, meta, cfg: Cfg, has_bias=False):
    import concourse.bass as bass  # noqa: F401
    from concourse import mybir

    nc = tc.nc
    f32 = mybir.dt.float32
    bf16 = mybir.dt.bfloat16
    i16 = mybir.dt.int16
    AF = mybir.ActivationFunctionType
    OP = mybir.AluOpType
    AX = mybir.AxisListType
    N, NSH, WIN, NWIN, SC = cfg.N, cfg.NSH, cfg.WIN, cfg.NWIN, cfg.SC
    TC, NG = meta["TC"], meta["NG"]
    cmap, nA = meta["cmap"], meta["nA"]
    NQ = cfg.NQUEUES
    R = cfg.R
    M1, M2 = cfg.MSG1, cfg.MSG2
    NIW = TC * cfg.CHUNK // 16

    ctx = ExitStack()
    with ctx:
        dram = ctx.enter_context(tc.tile_pool(name="dram", bufs=1, space="DRAM"))
        consts = ctx.enter_context(tc.tile_pool(name="consts", bufs=1))

        t1shard = dram.tile([NSH, R], bf16)
        t1full = dram.tile([N, R], bf16, addr_space="Shared")
        t2shard = dram.tile([NSH, R], bf16)
        t2full = dram.tile([N, R], bf16, addr_space="Shared")

        # ------- constants into SBUF -------
        W1_sb = consts.tile([128, 128], bf16)
        A1d_sb = consts.tile([128, 8], bf16)
        RHS2_sb = consts.tile([128, 65], f32)
        a1s_sb = consts.tile([128, 128], bf16)
        a2s_sb = consts.tile([128, 128], bf16)
        id_sb = consts.tile([128, 128], f32)
        iota_sb = consts.tile([128, 128], bf16)
        Qt_sb = consts.tile([128, 32], bf16)
        nc.sync.dma_start(W1_sb[:], ins["W1"][:])
        nc.sync.dma_start(A1d_sb[:], ins["A1d"][:])
        nc.sync.dma_start(RHS2_sb[:], ins["RHS2"][:])
        nc.sync.dma_start(a1s_sb[:], ins["a1s_rep"][:])
        nc.sync.dma_start(a2s_sb[:], ins["a2s_rep"][:])
        nc.sync.dma_start(id_sb[:], ins["ident"][:])
        nc.sync.dma_start(iota_sb[:], ins["iota_row"][:])
        nc.sync.dma_start(Qt_sb[:], ins["Qt"][:])
        src_sb = consts.tile([128, NIW], i16)
        rel_sb = consts.tile([128, TC], bf16)
        rel4_sb = consts.tile([128, TC * 4], bf16)
        nc.sync.dma_start(src_sb[:], ins["src_idx"][:])
        nc.sync.dma_start(rel_sb[:], ins["dst_rel"][:])
        nc.sync.dma_start(rel4_sb[:], ins["rel4"][:])
        if has_bias:
            b1rep_sb = consts.tile([128, 128], f32)
            b2rep_sb = consts.tile([128, 64], f32)
            nc.sync.dma_start(b1rep_sb[:], ins["b1rep"][:])
            nc.sync.dma_start(b2rep_sb[:], ins["b2rep"][:])

        # per-window dst logits, resident in SBUF for the whole kernel
        # (memset: the last window writes only 106 partitions; the matmul
        # contracts all 128, and 0 * garbage-Inf would poison the PSUM)
        aldW_sb = consts.tile([128, NWIN * 8], bf16)
        ald2_sb = consts.tile([128, NWIN], bf16)
        nc.vector.memset(aldW_sb[:], 0.0)
        nc.vector.memset(ald2_sb[:], 0.0)

        # ------- stage A: h1 rows -> table1, al_dst -> aldW_sb -------
        with tc.tile_pool(name="stageA", bufs=1) as sa, \
             tc.tile_pool(name="stageApsum", bufs=2, space="PSUM") as sap, \
             tc.tile_pool(name="rows", bufs=3) as rows:
            xT_sb = sa.tile([128, NSH], bf16)
            nc.sync.dma_start(xT_sb[:], ins["xT"][:])
            h1T_sb = sa.tile([128, NSH], bf16)
            al_sb = sa.tile([8, NSH], f32)
            nt = (NSH + 511) // 512
            for j in range(nt):
                w0 = j * 512
                w1 = min(NSH, w0 + 512)
                ph = sap.tile([128, 512], f32, tag="ph")
                nc.tensor.matmul(ph[:, : w1 - w0], W1_sb[:], xT_sb[:, w0:w1],
                                 start=True, stop=True)
                nc.vector.tensor_copy(h1T_sb[:, w0:w1], ph[:, : w1 - w0])
            for j in range(nt):
                w0 = j * 512
                w1 = min(NSH, w0 + 512)
                pa = sap.tile([8, 512], f32, tag="pa")
                nc.tensor.matmul(pa[:, : w1 - w0], A1d_sb[:], h1T_sb[:, w0:w1],
                                 start=True, stop=True)
                nc.vector.tensor_copy(al_sb[:, w0:w1], pa[:, : w1 - w0])

            for w in range(NWIN):
                w0 = w * WIN
                wn = min(WIN, NSH - w0)
                hp = sap.tile([128, 128], f32, tag="hp")
                nc.tensor.matmul(hp[:wn, :], xT_sb[:, w0:w0 + wn], W1_sb[:],
                                 start=True, stop=True)
                at = sap.tile([128, 8], f32, tag="at")
                nc.tensor.transpose(at[:wn, :], al_sb[:, w0:w0 + wn], id_sb[:8, :8])
                rowt = rows.tile([128, R], bf16, tag="rowt")
                nc.scalar.copy(rowt[:wn, :], hp[:wn, :])
                nc.sync.dma_start(t1shard.opt()[w0:w0 + wn, :], rowt[:wn, :])
                nc.vector.tensor_copy(aldW_sb[:wn, w * 8:(w + 1) * 8], at[:wn, :])

        from concourse import library_config

        nc.gpsimd.load_library(library_config.mlp)

        nc.gpsimd.collective_compute(
            "AllGather", mybir.AluOpType.bypass,
            replica_groups=[list(range(cfg.NCORES))],
            ins=[t1shard.opt()], outs=[t1full.opt()],
        )

        # ------- edge layers -------
        def edge_layer(table_full, nh, chper, msgc, a_rep_sb, ald_ap_of_w,
                       epilogue, qoff):
            with tc.tile_pool(name="gbuf", bufs=4) as gpool, \
                 tc.tile_pool(name="ebig", bufs=2) as epool, \
                 tc.tile_pool(name="emsg", bufs=3) as mp, \
                 tc.tile_pool(name="epsum", bufs=meta["max_inflight"] + 1,
                              space="PSUM") as pp, \
                 tc.tile_pool(name="aldpsum", bufs=2, space="PSUM") as app, \
                 tc.tile_pool(name="esmall", bufs=3) as spool:
                pw_by_w = {}
                for g in range(NG):
                    na = nA[g]
                    # rel broadcast materialized on ACT (idle capacity) so the
                    # s0 is_equal below runs all-step-1 (DVE 2x mode)
                    relw = epool.tile([128, SC * 128], bf16, tag="relw")
                    nc.scalar.copy(
                        _ap(relw[:], [[128, SC], [1, 128]]),
                        _ap(rel_sb[:, g * SC:(g + 1) * SC], [[1, SC], [0, 128]]),
                    )
                    gb = gpool.tile([128, SC * R], bf16, tag="gb")
                    gb3 = gb[:].rearrange("p (k e) -> p k e", k=SC)
                    c0 = g * SC * 8
                    for grp in range(2):
                        nch = na if grp == 0 else SC - na
                        if nch == 0:
                            continue
                        ksl = slice(0, na) if grp == 0 else slice(na, SC)
                        csl = (slice(c0, c0 + na * 8) if grp == 0
                               else slice(c0 + na * 8, c0 + SC * 8))
                        tbl = (table_full.opt() if grp == 0
                               else table_full.opt()[cfg.SPLIT:N, :])
                        nc.gpsimd.dma_gather(
                            gb3[:, ksl, :], tbl, src_sb[:, csl],
                            num_idxs=nch * 128, num_idxs_reg=nch * 128,
                            elem_size=R, single_packet=False,
                            queue_num=(g + qoff + 2 * grp) % NQ,
                        )
                    # src logits recomputed from the gathered rows
                    prod = epool.tile([128, SC * 128], bf16, tag="prod")
                    nc.vector.tensor_tensor(
                        _ap(prod[:], [[128, SC], [1, 128]]),
                        _ap(gb[:], [[128, SC], [1, 128]]),
                        _ap(a_rep_sb[:], [[0, SC], [1, 128]]),
                        OP.mult,
                    )
                    als = spool.tile([128, SC * nh], f32, tag="als")
                    nc.vector.tensor_reduce(
                        _ap(als[:], [[nh, SC], [1, nh]]),
                        _ap(prod[:], [[128, SC], [chper, nh], [1, chper]]),
                        AX.X, OP.add,
                    )
                    # transposed indicator S0T via block-swizzled is_equal +
                    # 32x32 stream transpose
                    xsw = epool.tile([128, SC * 128], bf16, tag="xsw")
                    nc.vector.tensor_tensor(
                        _ap(xsw[:], [[128, SC], [32, 4], [1, 32]]),
                        _ap(Qt_sb[:], [[0, SC], [0, 4], [1, 32]]),
                        _ap(rel4_sb[:, g * SC * 4:(g + 1) * SC * 4],
                            [[4, SC], [1, 4], [0, 32]]),
                        OP.is_equal,
                    )
                    s0T = epool.tile([128, SC * 128], bf16, tag="s0T")
                    nc.vector.transpose(s0T[:], xsw[:])
                    # per-edge dst logits: aldps[p, h] = sum_j S0T[j,p]*aldW[j,h]
                    aldps = app.tile([128, SC * nh], f32, tag="aldps")
                    for k in range(SC):
                        w = cmap[g * SC + k][0]
                        nc.tensor.matmul(
                            aldps[:, k * nh:(k + 1) * nh],
                            s0T[:, k * 128:(k + 1) * 128],
                            ald_ap_of_w(w),
                            start=True, stop=True,
                        )
                    lg = spool.tile([128, SC * nh], f32, tag="lg")
                    nc.vector.tensor_tensor(lg[:], als[:], aldps[:], OP.add)
                    lgm = spool.tile([128, SC * nh], f32, tag="lgm")
                    nc.vector.scalar_tensor_tensor(
                        lgm[:], lg[:], 0.2, lg[:], OP.mult, OP.max)
                    # exp with built-in replication across the channel block so
                    # the msg multiply runs all-step-1 (DVE 2x mode)
                    B = chper * nh
                    eew = epool.tile([128, SC * B], bf16, tag="eew")
                    nc.scalar.activation(
                        _ap(eew[:], [[B, SC], [1, B]]),
                        _ap(lgm[:], [[nh, SC], [1, nh], [0, chper]]),
                        AF.Exp)
                    msg = mp.tile([128, SC * msgc], bf16, tag="msg")
                    nc.vector.tensor_tensor(
                        _ap(msg[:], [[msgc, SC], [1, B]]),
                        _ap(gb[:], [[R, SC], [1, B]]),
                        _ap(eew[:], [[B, SC], [1, B]]),
                        OP.mult,
                    )
                    nc.scalar.copy(
                        _ap(msg[:], [[msgc, SC], [1, nh]], msgc - nh),
                        _ap(eew[:], [[B, SC], [chper, nh]]),
                    )
                    s0 = epool.tile([128, SC * 128], bf16, tag="s0")
                    nc.vector.tensor_tensor(
                        _ap(s0[:], [[128, SC], [1, 128]]),
                        _ap(iota_sb[:], [[0, SC], [1, 128]]),
                        _ap(relw[:], [[128, SC], [1, 128]]),
                        OP.is_equal,
                    )
                    for k in range(SC):
                        kk = g * SC + k
                        w, first, last = cmap[kk]
                        if first:
                            pw_by_w[w] = pp.tile([128, msgc], f32, tag="pw", name="pw")
                        pw = pw_by_w[w]
                        nc.tensor.matmul(
                            pw[:], s0[:, k * 128:(k + 1) * 128],
                            msg[:, k * msgc:(k + 1) * msgc],
                            start=first, stop=last,
                        )
                        if last:
                            epilogue(w, pw_by_w.pop(w))

        # ---- L1 ----
        with tc.tile_pool(name="epi1", bufs=2) as hq, \
             tc.tile_pool(name="epi1p", bufs=1, space="PSUM") as hpp:
            def epi1(w, pw):
                w0 = w * WIN
                wn = min(WIN, NSH - w0)
                dn = hq.tile([128, 8], f32, tag="dn")
                nc.vector.tensor_scalar(dn[:], pw[:, 128:136], 1e-16, None, OP.add)
                rcp = hq.tile([128, 8], f32, tag="rcp")
                nc.vector.reciprocal(rcp[:], dn[:])
                hb = hq.tile([128, 128], f32, tag="hb")
                nc.vector.tensor_tensor(
                    _ap(hb[:], [[16, 8], [1, 16]]),
                    _ap(pw[:], [[16, 8], [1, 16]]),
                    _ap(rcp[:], [[1, 8], [0, 16]]),
                    OP.mult,
                )
                if has_bias:
                    nc.vector.tensor_tensor(hb[:], hb[:], b1rep_sb[:], OP.add)
                nc.scalar.activation(hb[:], hb[:], AF.Relu)
                tp = hpp.tile([128, 128], f32, tag="tp")
                nc.tensor.transpose(tp[:], hb[:], id_sb[:])
                tH = hq.tile([128, 128], f32, tag="tH")
                nc.vector.tensor_copy(tH[:], tp[:])
                p2 = hpp.tile([128, 65], f32, tag="p2")
                nc.tensor.matmul(p2[:], tH[:], RHS2_sb[:], start=True, stop=True)
                t2b = hq.tile([128, R], bf16, tag="t2b")
                nc.scalar.copy(t2b[:wn, 0:64], p2[:wn, 1:65])
                nc.vector.memset(t2b[:wn, 64:R], 0.0)
                nc.sync.dma_start(t2shard.opt()[w0:w0 + wn, :], t2b[:wn, :])
                nc.vector.tensor_copy(ald2_sb[:wn, w:w + 1], p2[:wn, 0:1])

            edge_layer(t1full, 8, 16, M1, a1s_sb,
                       lambda w: aldW_sb[:, w * 8:(w + 1) * 8], epi1, 0)

        nc.gpsimd.collective_compute(
            "AllGather", mybir.AluOpType.bypass,
            replica_groups=[list(range(cfg.NCORES))],
            ins=[t2shard.opt()], outs=[t2full.opt()],
        )

        # ---- L2 ----
        with tc.tile_pool(name="epi2", bufs=2) as oq:
            def epi2(w, pw):
                w0 = w * WIN
                wn = min(WIN, NSH - w0)
                dn2 = oq.tile([128, 1], f32, tag="dn2")
                nc.vector.tensor_scalar(dn2[:], pw[:, 64:65], 1e-16, None, OP.add)
                rcp2 = oq.tile([128, 1], f32, tag="rcp2")
                nc.vector.reciprocal(rcp2[:], dn2[:])
                ob = oq.tile([128, 64], f32, tag="ob")
                nc.vector.tensor_scalar(ob[:], pw[:, 0:64], rcp2[:], None, OP.mult)
                if has_bias:
                    nc.vector.tensor_tensor(ob[:], ob[:], b2rep_sb[:], OP.add)
                nc.sync.dma_start(out_ap[w0:w0 + wn, :], ob[:wn, :])

            edge_layer(t2full, 1, 64, M2, a2s_sb,
                       lambda w: ald2_sb[:, w:w + 1], epi2, 1)


# ---------------------------------------------------------------------------
# SPMD build + run
# ---------------------------------------------------------------------------

_CACHE = {}


def _build(meta, cfg: Cfg, has_bias: bool):
    key = (tuple(meta["cmap"]), tuple(meta["nA"]), cfg.N, cfg.NCORES, has_bias)
    if key in _CACHE:
        return _CACHE[key]
    import concourse.tile as tile
    from concourse import bacc, mybir

    f32 = mybir.dt.float32
    bf16 = mybir.dt.bfloat16
    i16 = mybir.dt.int16
    TC = meta["TC"]
    NIW = TC * cfg.CHUNK // 16
    nc = bacc.Bacc("TRN2", target_bir_lowering=False, debug=False,
                   num_devices=cfg.NCORES,
                   num_swdge_queues=cfg.NQUEUES)
    ins = {}

    def di(name, shape, dt=f32):
        ins[name] = nc.dram_tensor(name, shape, dt, kind="ExternalInput").ap()

    di("xT", [128, cfg.NSH], bf16)
    di("W1", [128, 128], bf16)
    di("A1d", [128, 8], bf16)
    di("RHS2", [128, 65])
    di("a1s_rep", [128, 128], bf16)
    di("a2s_rep", [128, 128], bf16)
    di("ident", [128, 128])
    di("iota_row", [128, 128], bf16)
    di("Qt", [128, 32], bf16)
    di("src_idx", [128, NIW], i16)
    di("dst_rel", [128, TC], bf16)
    di("rel4", [128, TC * 4], bf16)
    if has_bias:
        di("b1rep", [128, 128])
        di("b2rep", [128, 64])
    out = nc.dram_tensor("out", [cfg.NSH, cfg.OUT_C], f32, kind="ExternalOutput").ap()

    with tile.TileContext(nc) as tc:
        emit_gat(tc, out, ins, meta, cfg, has_bias)
    nc.compile()
    _CACHE[key] = nc
    return nc


def kernel(**inputs) -> np.ndarray:
    out, _ = _run(inputs)
    return out


def _run(inputs, **run_kwargs):
    cfg = FULL
    inputs = {k: np.asarray(v) for k, v in inputs.items()}
    edge_index = inputs["edge_index"].astype(np.int64)
    meta, per_core = prep_edges(edge_index, cfg)
    shared, xT, has_bias = host_tensors(inputs, cfg)
    nc = _build(meta, cfg, has_bias)

    from concourse.bass_utils import run_bass_kernel_spmd

    in_maps = []
    for c in range(cfg.NCORES):
        m = {k: shared[k] for k in ("W1", "A1d", "RHS2", "a1s_rep", "a2s_rep",
                                    "ident", "iota_row", "Qt")}
        if has_bias:
            m["b1rep"] = shared["b1rep"]
            m["b2rep"] = shared["b2rep"]
        m["xT"] = xT[c]
        m.update(per_core[c])
        in_maps.append(m)
    res = run_bass_kernel_spmd(nc, in_maps, core_ids=list(range(cfg.NCORES)),
                               **run_kwargs)
    out = np.concatenate([res.results[c]["out"] for c in range(cfg.NCORES)], 0)
    return out.astype(np.float32), res


# revision 45
# speedup vs baseline: 1.0903x; 1.0564x over previous
"""Trainium2 Bass kernel for 2-layer GAT (nn_GAT_72619307041134).

Strategy (dst-sharded edge parallelism, 8 cores SPMD):
- Nodes sharded into 8 contiguous ranges of 6250; edges sorted by dst and
  sharded by dst range, so each core owns ALL edges of its dst nodes and the
  segment softmax + aggregation need no cross-core reduction.
- Per layer a per-node gather table lives in DRAM with 256 B rows (the
  dma_gather minimum): layer1 rows hold h1 (128 bf16), layer2 rows hold
  h2 (64 bf16 + zero pad).  The per-edge src logits al_src are NOT stored:
  they are recomputed on-chip from the gathered rows (DVE multiply by a
  replicated a_src row + grouped reduce), and the per-edge dst logits al_dst
  are expanded on-chip from a per-window SBUF table via a transposed-indicator
  PE matmul (no per-edge dst gather at all).  This halves layer-1 gather
  bytes and removes the 256 B/edge ald gathers entirely - the SWDGE gather
  queues (~40-60 GB/s aggregate drain, 4 queues max) are the bottleneck
  resource, with DVE a close second; both run ~92% busy in the edge phases.
- Edges are processed in 128-edge chunks (16 chunks = 1 super-chunk):
  dma_gather fetches the src rows (int16 indices; srcs >= SPLIT gather from a
  shifted view of the table; chunks are A/B-pure and A-first within each
  super-chunk); the two gathers of each super-chunk rotate over all 4 SWDGE
  queues.  DVE computes the logits; ACT exp writes a channel-replicated eew
  tile (step-0 input AP, step-1 output) so the big msg = ee*h multiply runs
  as an all-step-1 bf16 tensor_tensor in the DVE 2x mode.  An indicator
  S0[p,j] = (dst_rel[p]==j) turns the per-128-node-window segment-sum into PE
  matmuls accumulating in PSUM (bf16 operands -> FWL weight loads).  The
  transposed indicator S0T (for the al_dst expansion) is built by a
  block-swizzled is_equal followed by a single DVE 32x32 stream-transpose.
  Window epilogue divides by the softmax denominator (alpha = ee/(sum+1e-16);
  the segment-max shift is skipped because logits are bounded by ~4).
- Pad edges have dst_rel=-1 (S0/S0T columns all zero) and gather row 0.
- Between layers one AllGather shares each core's table shard.
- Host preprocessing only sorts/pads/permutes integer indices.
"""

from contextlib import ExitStack

import numpy as np

try:
    import ml_dtypes

    BF16 = ml_dtypes.bfloat16
except Exception:  # pragma: no cover
    BF16 = np.float32

# ---------------------------------------------------------------------------
# config
# ---------------------------------------------------------------------------


class Cfg:
    def __init__(self, N=50000, E=800000, NCORES=8, WIN=128, CHUNK=128, SC=16,
                 SPLIT=32768):
        self.N = N
        self.E = E
        self.NCORES = NCORES
        self.NSH = N // NCORES
        self.WIN = WIN
        self.NWIN = (self.NSH + WIN - 1) // WIN
        self.CHUNK = CHUNK
        self.SC = SC
        self.SPLIT = SPLIT        # src < SPLIT -> table A view, else B view
        self.HEADS = 8
        self.HID = 16
        self.OUT_C = 64
        self.R = 128              # bf16 elems per table row (256 B)
        self.MSG1 = 128 + 8
        self.MSG2 = 64 + 1
        self.NQUEUES = 4


FULL = Cfg()

# ---------------------------------------------------------------------------
# host-side edge preprocessing (indices only)
# ---------------------------------------------------------------------------


def prep_edges(edge_index: np.ndarray, cfg: Cfg):
    """Builds the SPMD-common chunk schedule and per-core index arrays.

    meta: cmap (slot -> (window, first, last)), nA (A-chunks per super-chunk,
    A-first slot order), TC, NG.
    per_core[c]: int16 'src_idx' [128, TC*CHUNK/16] (wrapped: edge i of the
    slot-ordered stream at [i%16, i//16], replicated to all 8 Q7 core pairs),
    bf16 'dst_rel' [128, TC] (rel[p, slot], -1 for pads) and bf16 'rel4'
    [128, TC*4] (quadrant-replicated rel for the transposed-indicator build:
    rel4[32q+r, slot*4+b] = rel[32b+r, slot]).
    """
    N, NSH, WIN, CHUNK, SC = cfg.N, cfg.NSH, cfg.WIN, cfg.CHUNK, cfg.SC
    loops = np.arange(N, dtype=np.int64)
    src = np.concatenate([edge_index[0].astype(np.int64), loops])
    dst = np.concatenate([edge_index[1].astype(np.int64), loops])
    order = np.argsort(dst, kind="stable")
    src = src[order]
    dst = dst[order]
    isA = src < cfg.SPLIT
    core_of = dst // NSH
    w_of = (dst % NSH) // WIN
    cntA = np.zeros((cfg.NCORES, cfg.NWIN), np.int64)
    cntB = np.zeros((cfg.NCORES, cfg.NWIN), np.int64)
    np.add.at(cntA, (core_of[isA], w_of[isA]), 1)
    np.add.at(cntB, (core_of[~isA], w_of[~isA]), 1)
    cpwA = np.ceil(cntA.max(0) / CHUNK).astype(np.int64)
    cpwB = np.ceil(cntB.max(0) / CHUNK).astype(np.int64)
    tc = int((cpwA + cpwB).sum())
    cpwA[-1] += (-tc) % SC
    TC = int((cpwA + cpwB).sum())
    NG = TC // SC

    # global chunk list: window-major, A then B inside a window
    glist = []  # (window, is_A)
    for w in range(cfg.NWIN):
        glist += [(w, True)] * int(cpwA[w]) + [(w, False)] * int(cpwB[w])
    # per-super-chunk stable reorder: A-chunks first
    slot_of = []
    nA = []
    for g in range(NG):
        blk = list(range(g * SC, (g + 1) * SC))
        a = [i for i in blk if glist[i][1]]
        b = [i for i in blk if not glist[i][1]]
        nA.append(len(a))
        slot_of += a + b
    lastslot = {}
    for s, gi in enumerate(slot_of):
        lastslot[glist[gi][0]] = s
    cmap = []
    seen = set()
    inflight = 0
    max_inflight = 0
    for s, gi in enumerate(slot_of):
        w = glist[gi][0]
        first = w not in seen
        seen.add(w)
        last = lastslot[w] == s
        if first:
            inflight += 1
            max_inflight = max(max_inflight, inflight)
        cmap.append((w, first, last))
        if last:
            inflight -= 1

    starts = np.searchsorted(
        dst, (np.arange(0, N, NSH)[:, None] + np.arange(0, NSH, WIN)[None, :]))
    flat_starts = list(starts.ravel()) + [len(dst)]
    per_core = []
    for c in range(cfg.NCORES):
        s_by_g = np.zeros((len(glist), CHUNK), np.int64)
        r_by_g = np.full((len(glist), CHUNK), -1, np.int64)
        gi = 0
        for w in range(cfg.NWIN):
            i = c * cfg.NWIN + w
            s0, s1 = flat_starts[i], flat_starts[i + 1]
            sl = src[s0:s1]
            dl = dst[s0:s1]
            il = isA[s0:s1]
            for grp in (True, False):
                m = il == grp
                se = sl[m]
                de = dl[m]
                nch = int(cpwA[w]) if grp else int(cpwB[w])
                buf_s = np.zeros(nch * CHUNK, np.int64)
                buf_r = np.full(nch * CHUNK, -1, np.int64)
                buf_s[:len(se)] = se - (0 if grp else cfg.SPLIT)
                buf_r[:len(se)] = de - (c * NSH + w * WIN)
                s_by_g[gi:gi + nch] = buf_s.reshape(nch, CHUNK)
                r_by_g[gi:gi + nch] = buf_r.reshape(nch, CHUNK)
                gi += nch
        svals = s_by_g[slot_of]   # [TC, 128] slot-ordered
        rvals = r_by_g[slot_of]

        def wrap(vals):
            st = vals.reshape(-1)   # stream i = slot*128 + p
            n = st.shape[0]
            out = np.zeros((16, n // 16), np.int16)
            idx = np.arange(n)
            out[idx % 16, idx // 16] = st
            return np.tile(out, (8, 1))

        relT = np.ascontiguousarray(rvals.T).astype(np.float32)  # [128, TC]
        # rel4[32q+r, t*4+b] = relT[32b+r, t]  (pre-swizzled rel for the
        # block-transposed indicator build)
        rel4q = relT.reshape(4, 32, TC).transpose(1, 2, 0).reshape(32, TC * 4)
        rel4 = np.tile(rel4q, (4, 1))
        per_core.append(dict(
            src_idx=wrap(svals),
            dst_rel=relT.astype(BF16),
            rel4=np.ascontiguousarray(rel4).astype(BF16),
        ))
    meta = dict(cmap=cmap, nA=nA, TC=TC, NG=NG, max_inflight=max_inflight)
    return meta, per_core


def host_tensors(inputs, cfg: Cfg):
    x = np.ascontiguousarray(inputs["x"], np.float32)
    W1 = np.ascontiguousarray(inputs["W1"], np.float32)
    a1s = np.asarray(inputs["a1_src"], np.float32)
    a1d = np.asarray(inputs["a1_dst"], np.float32)
    W2 = np.ascontiguousarray(inputs["W2"], np.float32)
    a2s = np.asarray(inputs["a2_src"], np.float32).reshape(1, -1)
    a2d = np.asarray(inputs["a2_dst"], np.float32).reshape(1, -1)
    b1 = np.asarray(inputs["b1"], np.float32)
    b2 = np.asarray(inputs["b2"], np.float32)
    H, HID = cfg.HEADS, cfg.HID
    A1d = np.zeros((H * HID, H), np.float32)
    for h in range(H):
        A1d[h * HID:(h + 1) * HID, h] = a1d[h]
    RHS2 = np.concatenate([W2 @ a2d.T, W2], 1).astype(np.float32)  # [128, 65]
    a1s_rep = np.tile(a1s.reshape(1, -1), (128, 1)).astype(BF16)   # [128,128]
    a2s_flat = np.zeros((1, 128), np.float32)
    a2s_flat[0, :cfg.OUT_C] = a2s[0]
    a2s_rep = np.tile(a2s_flat, (128, 1)).astype(BF16)             # [128,128]
    ident = np.eye(128, dtype=np.float32)
    iota_row = np.tile(np.arange(128, dtype=np.float32).reshape(1, -1),
                       (128, 1)).astype(BF16)                  # [p,j]=j
    p = np.arange(128)
    Qt = ((p // 32 * 32)[:, None] + (np.arange(128) % 32)[None, :]).astype(
        np.float32).astype(BF16)                               # [128,128]
    shared = dict(W1=W1.astype(BF16), A1d=A1d.astype(BF16), RHS2=RHS2,
                  a1s_rep=a1s_rep, a2s_rep=a2s_rep, ident=ident,
                  iota_row=iota_row, Qt=Qt,
                  b1rep=np.tile(b1.reshape(1, -1), (128, 1)),
                  b2rep=np.tile(b2.reshape(1, -1), (128, 1)))
    xT = [np.ascontiguousarray(x[c * cfg.NSH:(c + 1) * cfg.NSH].T).astype(BF16)
          for c in range(cfg.NCORES)]
    has_bias = bool(np.any(b1) or np.any(b2))
    return shared, xT, has_bias


# ---------------------------------------------------------------------------
# device kernel emission
# ---------------------------------------------------------------------------


def _ap(base, free_dims, extra_off=0):
    """Replace the free dims of a [P, ...] AP (keep partition dim)."""
    import concourse.bass as bass

    return bass.AP(base.tensor, base.offset + extra_off,
                   [list(base.ap[0])] + [list(d) for d in free_dims])


def emit_gat(tc, out_ap, ins, meta, cfg: Cfg, has_bias=False):
    import concourse.bass as bass  # noqa: F401
    from concourse import mybir

    nc = tc.nc
    f32 = mybir.dt.float32
    bf16 = mybir.dt.bfloat16
    i16 = mybir.dt.int16
    AF = mybir.ActivationFunctionType
    OP = mybir.AluOpType
    AX = mybir.AxisListType
    N, NSH, WIN, NWIN, SC = cfg.N, cfg.NSH, cfg.WIN, cfg.NWIN, cfg.SC
    TC, NG = meta["TC"], meta["NG"]
    cmap, nA = meta["cmap"], meta["nA"]
    NQ = cfg.NQUEUES
    R = cfg.R
    M1, M2 = cfg.MSG1, cfg.MSG2
    NIW = TC * cfg.CHUNK // 16

    ctx = ExitStack()
    with ctx:
        dram = ctx.enter_context(tc.tile_pool(name="dram", bufs=1, space="DRAM"))
        consts = ctx.enter_context(tc.tile_pool(name="consts", bufs=1))

        t1shard = dram.tile([NSH, R], bf16)
        t1full = dram.tile([N, R], bf16, addr_space="Shared")
        t2shard = dram.tile([NSH, R], bf16)
        t2full = dram.tile([N, R], bf16, addr_space="Shared")

        # ------- constants into SBUF -------
        W1_sb = consts.tile([128, 128], bf16)
        A1d_sb = consts.tile([128, 8], bf16)
        RHS2_sb = consts.tile([128, 65], f32)
        a1s_sb = consts.tile([128, 128], bf16)
        a2s_sb = consts.tile([128, 128], bf16)
        id_sb = consts.tile([128, 128], f32)
        iota_sb = consts.tile([128, 128], bf16)
        Qt_sb = consts.tile([128, 128], bf16)
        nc.sync.dma_start(W1_sb[:], ins["W1"][:])
        nc.sync.dma_start(A1d_sb[:], ins["A1d"][:])
        nc.sync.dma_start(RHS2_sb[:], ins["RHS2"][:])
        nc.sync.dma_start(a1s_sb[:], ins["a1s_rep"][:])
        nc.sync.dma_start(a2s_sb[:], ins["a2s_rep"][:])
        nc.sync.dma_start(id_sb[:], ins["ident"][:])
        nc.sync.dma_start(iota_sb[:], ins["iota_row"][:])
        nc.sync.dma_start(Qt_sb[:], ins["Qt"][:])
        src_sb = consts.tile([128, NIW], i16)
        rel_sb = consts.tile([128, TC], bf16)
        rel4_sb = consts.tile([128, TC * 4], bf16)
        nc.sync.dma_start(src_sb[:], ins["src_idx"][:])
        nc.sync.dma_start(rel_sb[:], ins["dst_rel"][:])
        nc.sync.dma_start(rel4_sb[:], ins["rel4"][:])
        if has_bias:
            b1rep_sb = consts.tile([128, 128], f32)
            b2rep_sb = consts.tile([128, 64], f32)
            nc.sync.dma_start(b1rep_sb[:], ins["b1rep"][:])
            nc.sync.dma_start(b2rep_sb[:], ins["b2rep"][:])

        # per-window dst logits, resident in SBUF for the whole kernel
        # (memset: the last window writes only 106 partitions; the matmul
        # contracts all 128, and 0 * garbage-Inf would poison the PSUM)
        aldW_sb = consts.tile([128, NWIN * 8], bf16)
        ald2_sb = consts.tile([128, NWIN], bf16)
        nc.vector.memset(aldW_sb[:], 0.0)
        nc.vector.memset(ald2_sb[:], 0.0)

        # ------- stage A: h1 rows -> table1, al_dst -> aldW_sb -------
        with tc.tile_pool(name="stageA", bufs=1) as sa, \
             tc.tile_pool(name="stageApsum", bufs=2, space="PSUM") as sap, \
             tc.tile_pool(name="rows", bufs=3) as rows:
            xT_sb = sa.tile([128, NSH], bf16)
            nc.sync.dma_start(xT_sb[:], ins["xT"][:])
            h1T_sb = sa.tile([128, NSH], bf16)
            al_sb = sa.tile([8, NSH], f32)
            nt = (NSH + 511) // 512
            for j in range(nt):
                w0 = j * 512
                w1 = min(NSH, w0 + 512)
                ph = sap.tile([128, 512], f32, tag="ph")
                nc.tensor.matmul(ph[:, : w1 - w0], W1_sb[:], xT_sb[:, w0:w1],
                                 start=True, stop=True)
                nc.vector.tensor_copy(h1T_sb[:, w0:w1], ph[:, : w1 - w0])
            for j in range(nt):
                w0 = j * 512
                w1 = min(NSH, w0 + 512)
                pa = sap.tile([8, 512], f32, tag="pa")
                nc.tensor.matmul(pa[:, : w1 - w0], A1d_sb[:], h1T_sb[:, w0:w1],
                                 start=True, stop=True)
                nc.vector.tensor_copy(al_sb[:, w0:w1], pa[:, : w1 - w0])

            for w in range(NWIN):
                w0 = w * WIN
                wn = min(WIN, NSH - w0)
                hp = sap.tile([128, 128], f32, tag="hp")
                nc.tensor.matmul(hp[:wn, :], xT_sb[:, w0:w0 + wn], W1_sb[:],
                                 start=True, stop=True)
                at = sap.tile([128, 8], f32, tag="at")
                nc.tensor.transpose(at[:wn, :], al_sb[:, w0:w0 + wn], id_sb[:8, :8])
                rowt = rows.tile([128, R], bf16, tag="rowt")
                nc.scalar.copy(rowt[:wn, :], hp[:wn, :])
                nc.sync.dma_start(t1shard.opt()[w0:w0 + wn, :], rowt[:wn, :])
                nc.vector.tensor_copy(aldW_sb[:wn, w * 8:(w + 1) * 8], at[:wn, :])

        from concourse import library_config

        nc.gpsimd.load_library(library_config.mlp)

        nc.gpsimd.collective_compute(
            "AllGather", mybir.AluOpType.bypass,
            replica_groups=[list(range(cfg.NCORES))],
            ins=[t1shard.opt()], outs=[t1full.opt()],
        )

        # ------- edge layers -------
        def edge_layer(table_full, nh, chper, msgc, a_rep_sb, ald_ap_of_w,
                       epilogue, qoff):
            with tc.tile_pool(name="gbuf", bufs=4) as gpool, \
                 tc.tile_pool(name="ebig", bufs=2) as epool, \
                 tc.tile_pool(name="emsg", bufs=3) as mp, \
                 tc.tile_pool(name="epsum", bufs=meta["max_inflight"] + 1,
                              space="PSUM") as pp, \
                 tc.tile_pool(name="aldpsum", bufs=2, space="PSUM") as app, \
                 tc.tile_pool(name="esmall", bufs=3) as spool:
                pw_by_w = {}
                for g in range(NG):
                    na = nA[g]
                    # rel broadcast materialized on ACT (idle capacity) so the
                    # s0 is_equal below runs all-step-1 (DVE 2x mode)
                    relw = epool.tile([128, SC * 128], bf16, tag="relw")
                    nc.scalar.copy(
                        _ap(relw[:], [[128, SC], [1, 128]]),
                        _ap(rel_sb[:, g * SC:(g + 1) * SC], [[1, SC], [0, 128]]),
                    )
                    gb = gpool.tile([128, SC * R], bf16, tag="gb")
                    gb3 = gb[:].rearrange("p (k e) -> p k e", k=SC)
                    c0 = g * SC * 8
                    for grp in range(2):
                        nch = na if grp == 0 else SC - na
                        if nch == 0:
                            continue
                        ksl = slice(0, na) if grp == 0 else slice(na, SC)
                        csl = (slice(c0, c0 + na * 8) if grp == 0
                               else slice(c0 + na * 8, c0 + SC * 8))
                        tbl = (table_full.opt() if grp == 0
                               else table_full.opt()[cfg.SPLIT:N, :])
                        nc.gpsimd.dma_gather(
                            gb3[:, ksl, :], tbl, src_sb[:, csl],
                            num_idxs=nch * 128, num_idxs_reg=nch * 128,
                            elem_size=R, single_packet=False,
                            queue_num=(g + qoff + 2 * grp) % NQ,
                        )
                    # src logits recomputed from the gathered rows
                    prod = epool.tile([128, SC * 128], bf16, tag="prod")
                    nc.vector.tensor_tensor(
                        _ap(prod[:], [[128, SC], [1, 128]]),
                        _ap(gb[:], [[128, SC], [1, 128]]),
                        _ap(a_rep_sb[:], [[0, SC], [1, 128]]),
                        OP.mult,
                    )
                    als = spool.tile([128, SC * nh], f32, tag="als")
                    nc.vector.tensor_reduce(
                        _ap(als[:], [[nh, SC], [1, nh]]),
                        _ap(prod[:], [[128, SC], [chper, nh], [1, chper]]),
                        AX.X, OP.add,
                    )
                    # transposed indicator S0T via block-swizzled is_equal +
                    # 32x32 stream transpose; the swizzled rel operand is
                    # materialized on ACT so the is_equal runs all-step-1
                    rel4w = epool.tile([128, SC * 128], bf16, tag="rel4w")
                    nc.scalar.copy(
                        _ap(rel4w[:], [[128, SC], [32, 4], [1, 32]]),
                        _ap(rel4_sb[:, g * SC * 4:(g + 1) * SC * 4],
                            [[4, SC], [1, 4], [0, 32]]),
                    )
                    xsw = epool.tile([128, SC * 128], bf16, tag="xsw")
                    nc.vector.tensor_tensor(
                        _ap(xsw[:], [[128, SC], [1, 128]]),
                        _ap(Qt_sb[:], [[0, SC], [1, 128]]),
                        _ap(rel4w[:], [[128, SC], [1, 128]]),
                        OP.is_equal,
                    )
                    s0T = epool.tile([128, SC * 128], bf16, tag="s0T")
                    nc.vector.transpose(s0T[:], xsw[:])
                    # per-edge dst logits: aldps[p, h] = sum_j S0T[j,p]*aldW[j,h]
                    aldps = app.tile([128, SC * nh], f32, tag="aldps")
                    for k in range(SC):
                        w = cmap[g * SC + k][0]
                        nc.tensor.matmul(
                            aldps[:, k * nh:(k + 1) * nh],
                            s0T[:, k * 128:(k + 1) * 128],
                            ald_ap_of_w(w),
                            start=True, stop=True,
                        )
                    lg = spool.tile([128, SC * nh], f32, tag="lg")
                    nc.vector.tensor_tensor(lg[:], als[:], aldps[:], OP.add)
                    lgm = spool.tile([128, SC * nh], f32, tag="lgm")
                    nc.vector.scalar_tensor_tensor(
                        lgm[:], lg[:], 0.2, lg[:], OP.mult, OP.max)
                    # exp with built-in replication across the channel block so
                    # the msg multiply runs all-step-1 (DVE 2x mode)
                    B = chper * nh
                    eew = epool.tile([128, SC * B], bf16, tag="eew")
                    nc.scalar.activation(
                        _ap(eew[:], [[B, SC], [1, B]]),
                        _ap(lgm[:], [[nh, SC], [1, nh], [0, chper]]),
                        AF.Exp)
                    msg = mp.tile([128, SC * msgc], bf16, tag="msg")
                    nc.vector.tensor_tensor(
                        _ap(msg[:], [[msgc, SC], [1, B]]),
                        _ap(gb[:], [[R, SC], [1, B]]),
                        _ap(eew[:], [[B, SC], [1, B]]),
                        OP.mult,
                    )
                    nc.scalar.copy(
                        _ap(msg[:], [[msgc, SC], [1, nh]], msgc - nh),
                        _ap(eew[:], [[B, SC], [chper, nh]]),
                    )
                    s0 = epool.tile([128, SC * 128], bf16, tag="s0")
                    nc.vector.tensor_tensor(
                        _ap(s0[:], [[128, SC], [1, 128]]),
                        _ap(iota_sb[:], [[0, SC], [1, 128]]),
                        _ap(relw[:], [[128, SC], [1, 128]]),
                        OP.is_equal,
                    )
                    for k in range(SC):
                        kk = g * SC + k
                        w, first, last = cmap[kk]
                        if first:
                            pw_by_w[w] = pp.tile([128, msgc], f32, tag="pw", name="pw")
                        pw = pw_by_w[w]
                        nc.tensor.matmul(
                            pw[:], s0[:, k * 128:(k + 1) * 128],
                            msg[:, k * msgc:(k + 1) * msgc],
                            start=first, stop=last,
                        )
                        if last:
                            epilogue(w, pw_by_w.pop(w))

        # ---- L1 ----
        with tc.tile_pool(name="epi1", bufs=2) as hq, \
             tc.tile_pool(name="epi1p", bufs=1, space="PSUM") as hpp:
            def epi1(w, pw):
                w0 = w * WIN
                wn = min(WIN, NSH - w0)
                dn = hq.tile([128, 8], f32, tag="dn")
                nc.vector.tensor_scalar(dn[:], pw[:, 128:136], 1e-16, None, OP.add)
                rcp = hq.tile([128, 8], f32, tag="rcp")
                nc.vector.reciprocal(rcp[:], dn[:])
                hb = hq.tile([128, 128], f32, tag="hb")
                nc.vector.tensor_tensor(
                    _ap(hb[:], [[16, 8], [1, 16]]),
                    _ap(pw[:], [[16, 8], [1, 16]]),
                    _ap(rcp[:], [[1, 8], [0, 16]]),
                    OP.mult,
                )
                if has_bias:
                    nc.vector.tensor_tensor(hb[:], hb[:], b1rep_sb[:], OP.add)
                nc.scalar.activation(hb[:], hb[:], AF.Relu)
                tp = hpp.tile([128, 128], f32, tag="tp")
                nc.tensor.transpose(tp[:], hb[:], id_sb[:])
                tH = hq.tile([128, 128], f32, tag="tH")
                nc.vector.tensor_copy(tH[:], tp[:])
                p2 = hpp.tile([128, 65], f32, tag="p2")
                nc.tensor.matmul(p2[:], tH[:], RHS2_sb[:], start=True, stop=True)
                t2b = hq.tile([128, R], bf16, tag="t2b")
                nc.scalar.copy(t2b[:wn, 0:64], p2[:wn, 1:65])
                nc.vector.memset(t2b[:wn, 64:R], 0.0)
                nc.sync.dma_start(t2shard.opt()[w0:w0 + wn, :], t2b[:wn, :])
                nc.vector.tensor_copy(ald2_sb[:wn, w:w + 1], p2[:wn, 0:1])

            edge_layer(t1full, 8, 16, M1, a1s_sb,
                       lambda w: aldW_sb[:, w * 8:(w + 1) * 8], epi1, 0)

        nc.gpsimd.collective_compute(
            "AllGather", mybir.AluOpType.bypass,
            replica_groups=[list(range(cfg.NCORES))],
            ins=[t2shard.opt()], outs=[t2full.opt()],
        )

        # ---- L2 ----
        with tc.tile_pool(name="epi2", bufs=2) as oq:
            def epi2(w, pw):
                w0 = w * WIN
                wn = min(WIN, NSH - w0)
                dn2 = oq.tile([128, 1], f32, tag="dn2")
                nc.vector.tensor_scalar(dn2[:], pw[:, 64:65], 1e-16, None, OP.add)
                rcp2 = oq.tile([128, 1], f32, tag="rcp2")
                nc.vector.reciprocal(rcp2[:], dn2[:])
                ob = oq.tile([128, 64], f32, tag="ob")
                nc.vector.tensor_scalar(ob[:], pw[:, 0:64], rcp2[:], None, OP.mult)
                if has_bias:
                    nc.vector.tensor_tensor(ob[:], ob[:], b2rep_sb[:], OP.add)
                nc.sync.dma_start(out_ap[w0:w0 + wn, :], ob[:wn, :])

            edge_layer(t2full, 1, 64, M2, a2s_sb,
                       lambda w: ald2_sb[:, w:w + 1], epi2, 1)


# ---------------------------------------------------------------------------
# SPMD build + run
# ---------------------------------------------------------------------------

_CACHE = {}


def _build(meta, cfg: Cfg, has_bias: bool):
    key = (tuple(meta["cmap"]), tuple(meta["nA"]), cfg.N, cfg.NCORES, has_bias)
    if key in _CACHE:
        return _CACHE[key]
    import concourse.tile as tile
    from concourse import bacc, mybir

    f32 = mybir.dt.float32
    bf16 = mybir.dt.bfloat16
    i16 = mybir.dt.int16
    TC = meta["TC"]
    NIW = TC * cfg.CHUNK // 16
    nc = bacc.Bacc("TRN2", target_bir_lowering=False, debug=False,
                   num_devices=cfg.NCORES,
                   num_swdge_queues=cfg.NQUEUES)
    ins = {}

    def di(name, shape, dt=f32):
        ins[name] = nc.dram_tensor(name, shape, dt, kind="ExternalInput").ap()

    di("xT", [128, cfg.NSH], bf16)
    di("W1", [128, 128], bf16)
    di("A1d", [128, 8], bf16)
    di("RHS2", [128, 65])
    di("a1s_rep", [128, 128], bf16)
    di("a2s_rep", [128, 128], bf16)
    di("ident", [128, 128])
    di("iota_row", [128, 128], bf16)
    di("Qt", [128, 128], bf16)
    di("src_idx", [128, NIW], i16)
    di("dst_rel", [128, TC], bf16)
    di("rel4", [128, TC * 4], bf16)
    if has_bias:
        di("b1rep", [128, 128])
        di("b2rep", [128, 64])
    out = nc.dram_tensor("out", [cfg.NSH, cfg.OUT_C], f32, kind="ExternalOutput").ap()

    with tile.TileContext(nc) as tc:
        emit_gat(tc, out, ins, meta, cfg, has_bias)
    nc.compile()
    _CACHE[key] = nc
    return nc


def kernel(**inputs) -> np.ndarray:
    out, _ = _run(inputs)
    return out


def _run(inputs, **run_kwargs):
    cfg = FULL
    inputs = {k: np.asarray(v) for k, v in inputs.items()}
    edge_index = inputs["edge_index"].astype(np.int64)
    meta, per_core = prep_edges(edge_index, cfg)
    shared, xT, has_bias = host_tensors(inputs, cfg)
    nc = _build(meta, cfg, has_bias)

    from concourse.bass_utils import run_bass_kernel_spmd

    in_maps = []
    for c in range(cfg.NCORES):
        m = {k: shared[k] for k in ("W1", "A1d", "RHS2", "a1s_rep", "a2s_rep",
                                    "ident", "iota_row", "Qt")}
        if has_bias:
            m["b1rep"] = shared["b1rep"]
            m["b2rep"] = shared["b2rep"]
        m["xT"] = xT[c]
        m.update(per_core[c])
        in_maps.append(m)
    res = run_bass_kernel_spmd(nc, in_maps, core_ids=list(range(cfg.NCORES)),
                               **run_kwargs)
    out = np.concatenate([res.results[c]["out"] for c in range(cfg.NCORES)], 0)
    return out.astype(np.float32), res
